# revision 7
# baseline (speedup 1.0000x reference)
"""Binary associative memory (causal linear attention with binarized k/v).

Self-contained Trainium2 Bass kernel.

Math: the reference's chunked prefix recurrence telescopes to exact causal
linear attention:
    out[t] = (1/(8*(t+1))) * sum_{s<=t} (q[t].k[s]) v[s],   k,v = sign(qkv)
    y      = out @ W_o.T   (summed over head features)
    final_matrix[b,h] = sum_t k[t] (x) v[t]   (exact integers)
so we are free to re-chunk at 128 tokens (partition width).

Sharding: 8 cores = 4 batches x 2 head-groups (8 heads each). Each core:
  - feat-partition qkv projection (fp32r matmuls, N=512: full PE speed)
  - binarize k,v with ScalarE Sign (exact +-1), cast q to fp16
  - per 128-token chunk: scoresT/intraT/crossT/ckv matmuls in fp16
    (k,v exact in fp16; running state S holds exact small integers)
  - o_proj in fp32r; host sums the two partial y's per batch.
"""

import functools

import numpy as np

T = 4096
D = 1024
HLOC = 8  # heads per core
DH = 64
CH = 128  # our chunk size
NCH = T // CH  # 32
TT = 512  # projection token tile
NTT = T // TT  # 8
NFB = 12  # 1536 projection feature columns / 128


@functools.lru_cache(maxsize=1)
def _build():
    from contextlib import ExitStack

    import concourse.bacc as bacc
    import concourse.mybir as mybir
    import concourse.tile as tile

    f32 = mybir.dt.float32
    f32r = mybir.dt.float32r
    f16 = mybir.dt.float16

    nc = bacc.Bacc("TRN2", target_bir_lowering=False, debug=False, num_devices=8)

    xT = nc.dram_tensor("xT", [D, T], f32r, kind="ExternalInput").ap()
    wqkvT = nc.dram_tensor("wqkvT", [D, 1536], f32r, kind="ExternalInput").ap()
    woT = nc.dram_tensor("woT", [512, D], f32r, kind="ExternalInput").ap()
    invtot = nc.dram_tensor("invtot", [128, T], f32, kind="ExternalInput").ap()
    mask8 = nc.dram_tensor("mask8", [128, 1024], f32, kind="ExternalInput").ap()
    ident = nc.dram_tensor("ident", [128, 128], f16, kind="ExternalInput").ap()
    ypart = nc.dram_tensor("ypart", [T, D], f32, kind="ExternalOutput").ap()
    fmat = nc.dram_tensor("fmat", [HLOC, DH, DH], f32, kind="ExternalOutput").ap()

    with tile.TileContext(nc) as tc, ExitStack() as ctx:
        const = ctx.enter_context(tc.tile_pool(name="const", bufs=1))
        xpool = ctx.enter_context(tc.tile_pool(name="xp", bufs=2))
        qkvpool = ctx.enter_context(tc.tile_pool(name="qkv", bufs=2))
        tokpool = ctx.enter_context(tc.tile_pool(name="tok", bufs=3))
        stpool = ctx.enter_context(tc.tile_pool(name="st", bufs=8))
        opool = ctx.enter_context(tc.tile_pool(name="op", bufs=3))
        ypool = ctx.enter_context(tc.tile_pool(name="yp", bufs=3))
        spool = ctx.enter_context(tc.tile_pool(name="sp", bufs=2))
        pspool = ctx.enter_context(tc.tile_pool(name="ps", bufs=8, space="PSUM"))

        # --- constants / weights ---
        wq_sb = const.tile([128, 8, 1536], f32r)
        wqv = wqkvT.rearrange("(kc p) f -> p kc f", p=128)
        for kc in range(8):  # split so first matmuls start early
            nc.sync.dma_start(wq_sb[:, kc, :], wqv[:, kc, :])
        wo_sb = const.tile([128, 4, 1024], f32r)
        nc.sync.dma_start(wo_sb, woT.rearrange("(jc p) i -> p jc i", p=128))
        inv_sb = const.tile([128, T], f32)
        nc.sync.dma_start(inv_sb, invtot)
        mask_sb = const.tile([128, 1024], f32)
        nc.sync.dma_start(mask_sb, mask8)
        id_sb = const.tile([128, 128], f16)
        nc.sync.dma_start(id_sb, ident)

        # --- running state S[kf, df], packed [128, 4*64] fp16 (exact ints) ---
        s_cur = spool.tile([128, 256], f16, tag="S", name="s_init")
        nc.vector.memset(s_cur, 0.0)

        for tt in range(NTT):
            t0 = tt * TT
            # x slice, K-chunk-major: [128, kc, 512]
            xt = xpool.tile([128, 8, TT], f32r, tag="xt")
            xv = xT.rearrange("(kc p) t -> p kc t", p=128)
            for kc in range(8):
                nc.sync.dma_start(xt[:, kc, :], xv[:, kc, t0 : t0 + TT])

            # qkv projection (feature-partition): psum [feat 128, tok 512]
            qT = qkvpool.tile([128, 4 * TT], f16, tag="qT")
            kT = qkvpool.tile([128, 4 * TT], f16, tag="kT")
            vT = qkvpool.tile([128, 4 * TT], f16, tag="vT")
            for fb in range(NFB):
                pp = pspool.tile([128, 512], f32, tag="work", name="pp")
                for kc in range(8):
                    nc.tensor.matmul(
                        pp,
                        wq_sb[:, kc, fb * 128 : (fb + 1) * 128],
                        xt[:, kc, :],
                        start=(kc == 0),
                        stop=(kc == 7),
                    )
                if fb < 4:
                    nc.scalar.copy(qT[:, fb * TT : (fb + 1) * TT], pp)
                elif fb < 8:
                    nc.scalar.sign(kT[:, (fb - 4) * TT : (fb - 3) * TT], pp)
                else:
                    nc.scalar.sign(vT[:, (fb - 8) * TT : (fb - 7) * TT], pp)

            for ci in range(4):
                n = tt * 4 + ci
                c0 = ci * CH
                # --- scoresT[j, i], masked -> fp16 SBUF ---
                # Heads sharing a PSUM tile must use the SAME PE row-group
                # (same lhsT base partition): different-row-group matmuls run
                # concurrently in the array and same-bank writes collide
                # fatally. Pairs (0,2),(1,3),(4,6),(5,7).
                sts = []
                for p in range(4):
                    pair = [(0, 2), (1, 3), (4, 6), (5, 7)][p]
                    sc = pspool.tile([128, 256], f32, tag="work", name="sc")
                    for idx, h in enumerate(pair):
                        bp = (h % 2) * 64
                        hp = h // 2
                        sl = slice(hp * TT + c0, hp * TT + c0 + CH)
                        nc.tensor.matmul(
                            sc[:, idx * 128 : (idx + 1) * 128],
                            kT[bp : bp + 64, sl],
                            qT[bp : bp + 64, sl],
                            start=True,
                            stop=True,
                        )
                    st = stpool.tile([128, 256], f16, tag="st")
                    nc.vector.tensor_mul(st, sc, mask_sb[:, :256])
                    sts.append(st)

                # --- transpose k,v head-pair blocks -> token-partition ---
                ktp = pspool.tile([128, 512], f16, tag="work", name="ktp")
                vtp = pspool.tile([128, 512], f16, tag="work", name="vtp")
                for hp in range(4):
                    sl = slice(hp * TT + c0, hp * TT + c0 + CH)
                    nc.tensor.transpose(ktp[:, hp * 128 : (hp + 1) * 128], kT[:, sl], id_sb)
                    nc.tensor.transpose(vtp[:, hp * 128 : (hp + 1) * 128], vT[:, sl], id_sb)
                ktok = tokpool.tile([128, 512], f16, tag="ktok")
                vtok = tokpool.tile([128, 512], f16, tag="vtok")
                nc.scalar.copy(ktok, ktp)
                nc.scalar.copy(vtok, vtp)

                # --- ckv[kf, df] per head (outer product over chunk) ---
                ckv = pspool.tile([128, 256], f32, tag="work", name="ckv")
                for h in range(8):
                    bp = (h % 2) * 64
                    nc.tensor.matmul(
                        ckv[bp : bp + 64, (h // 2) * 64 : (h // 2) * 64 + 64],
                        ktok[:, h * 64 : h * 64 + 64],
                        vtok[:, h * 64 : h * 64 + 64],
                        start=True,
                        stop=True,
                        tile_position=(0, bp),
                    )

                # --- outT[dd, i] = intraT + crossT ---
                ot = pspool.tile([128, 512], f32, tag="work", name="ot")
                for h in range(8):
                    bp = (h % 2) * 64
                    hp = h // 2
                    sp_ = (h % 2) + 2 * (h // 4)
                    si_ = (h // 2) % 2
                    nc.tensor.matmul(
                        ot[bp : bp + 64, hp * 128 : hp * 128 + 128],
                        vtok[:, h * 64 : h * 64 + 64],
                        sts[sp_][:, si_ * 128 : si_ * 128 + 128],
                        start=True,
                        stop=(n == 0),
                        tile_position=(0, bp),
                    )
                    if n > 0:
                        nc.tensor.matmul(
                            ot[bp : bp + 64, hp * 128 : hp * 128 + 128],
                            s_cur[bp : bp + 64, hp * 64 : hp * 64 + 64],
                            qT[bp : bp + 64, hp * TT + c0 : hp * TT + c0 + CH],
                            start=False,
                            stop=True,
                            tile_position=(bp, bp),
                        )

                # --- state update S += ckv (exact ints in fp16) ---
                s_new = spool.tile([128, 256], f16, tag="S", name="s_new")
                nc.vector.tensor_add(s_new, s_cur, ckv)
                s_cur = s_new

                # --- scale by 1/(8*(t+1)), evacuate ---
                osb = opool.tile([128, 512], f32r, tag="osb")
                for hp in range(4):
                    nc.vector.tensor_mul(
                        osb[:, hp * 128 : (hp + 1) * 128],
                        ot[:, hp * 128 : (hp + 1) * 128],
                        inv_sb[:, n * CH : n * CH + CH],
                    )

                # --- o_proj: y[t, i] (partial over this core's 512 j-feats) ---
                ysb = ypool.tile([128, 1024], f32, tag="ysb")
                for icol in range(2):
                    yp = pspool.tile([128, 512], f32, tag="work", name="yp")
                    for hp in range(4):
                        nc.tensor.matmul(
                            yp,
                            osb[:, hp * 128 : (hp + 1) * 128],
                            wo_sb[:, hp, icol * 512 : (icol + 1) * 512],
                            start=(hp == 0),
                            stop=(hp == 3),
                        )
                    nc.scalar.copy(ysb[:, icol * 512 : (icol + 1) * 512], yp)
                nc.sync.dma_start(ypart[n * CH : (n + 1) * CH, :], ysb)

        # --- final matrix (exact integer sums) ---
        fsb = opool.tile([128, 256], f32, tag="fsb")
        nc.vector.tensor_copy(fsb, s_cur)
        for h in range(8):
            nc.sync.dma_start(
                fmat[h],
                fsb[(h % 2) * 64 : (h % 2) * 64 + 64, (h // 2) * 64 : (h // 2) * 64 + 64],
            )

    nc.compile()
    return nc


def _host_inputs(x, W_qkv, W_o):
    f32 = np.float32
    tvec = np.arange(1, T + 1, dtype=np.float64)
    inv = (1.0 / (8.0 * tvec)).astype(f32)
    invtot = np.ascontiguousarray(np.broadcast_to(inv[None, :], (128, T)))
    jj = np.arange(128)
    maskT = (jj[:, None] <= jj[None, :]).astype(f32)
    mask8 = np.ascontiguousarray(np.tile(maskT, (1, 8)))
    ident = np.eye(128, dtype=np.float16)

    Wq3 = np.asarray(W_qkv, dtype=f32).reshape(3, 16, DH, D)
    in_maps = []
    for core in range(8):
        b, g = core // 2, core % 2
        xTb = np.ascontiguousarray(np.asarray(x[b], dtype=f32).T)
        wsel = np.ascontiguousarray(
            Wq3[:, g * 8 : (g + 1) * 8].reshape(1536, D).T
        )  # [D, 1536]
        woT = np.ascontiguousarray(
            np.asarray(W_o, dtype=f32)[:, g * 512 : (g + 1) * 512].T
        )  # [512, D]
        in_maps.append(
            {
                "xT": xTb,
                "wqkvT": wsel,
                "woT": woT,
                "invtot": invtot,
                "mask8": mask8,
                "ident": ident,
            }
        )
    return in_maps


def kernel(x, W_qkv, W_o, trace=False):
    from concourse import bass_utils

    nc = _build()
    in_maps = _host_inputs(x, W_qkv, W_o)
    res = bass_utils.run_bass_kernel_spmd(
        nc, in_maps, core_ids=list(range(8)), trace=trace
    )
    results = res.results

    f32 = np.float32
    y = np.empty((4, T, D), dtype=f32)
    fm = np.empty((4, 16, DH, DH), dtype=f32)
    for core in range(8):
        b, g = core // 2, core % 2
        if g == 0:
            y[b] = results[core]["ypart"]
        else:
            y[b] += results[core]["ypart"]
        fm[b, g * 8 : (g + 1) * 8] = results[core]["fmat"]
    fc = np.full((4, 16, 1, 1), float(T), dtype=f32)
    if trace:
        kernel._last_result = res
    return (y, fm, fc)
